# revision 42
# baseline (speedup 1.0000x reference)
"""Trainium2 Bass kernel for nn_BottleneckR (pre-activation ternary bottleneck).

Reference (batch 64):
  h  = conv1x1(BN1(x), tern(w1))            [64,256,28,28]
  h  = conv3x3s2p1(BN2(h), tern(w2))        [64,256,14,14]
  h  = conv1x1(BN3(h), tern(w3))            [64,1024,14,14]
  sc = BN_ds(conv1x1s2(x, ds_w))            [64,1024,14,14]
  out = h + sc

Strategy: data-parallel over batch on 8 NeuronCores (8 images/core).
Exact sync-BN via 3 small AllReduces of per-channel sum/sumsq.

Differences vs the v1 kernel (306us):
 - bf16 end to end: x is converted to bf16 on the host (halves the HBM
   stream), all matmuls run bf16 at full PE rate, DVE elementwise ops get
   the 2x 16-bit mode. Ternary sign matrices are exact in bf16.
 - x stays resident in SBUF (one HBM read instead of two).
 - p1 is stored in a 29x29 zero-padded per-image layout; conv2's 9 taps
   read it directly with stride-2 access patterns (no phase-split scatter,
   no shifted-plane copies).
 - BN2 is folded into conv2 instead of materialized: the scale a2f folds
   into the t2 weights (per input channel), the shift b2f is handled by
   adding T[o] = sum_i S2sum[i,o]*b2f[i] at the psum->p2 copy while the
   pad cells hold -b2f/a2f (so border taps contribute zero shift, exactly
   matching zero padding after BN).
 - AR1 is triggered as soon as the x statistics are done; the ds-conv
   tail, q sum-of-squares and all q/p1 copies overlap the collective.
 - q's BN affine runs during AR3's latency window.
 - One persistent tile pool: no mid-kernel pool-exit barriers (the v1
   kernel lost ~16us of DVE time to one of those after AR3).
"""

import sys

sys.path.insert(0, "/opt/trn_rl_repo")

import numpy as np
import ml_dtypes

import concourse.bacc as bacc
import concourse.mybir as mybir
import concourse.tile as tile
from concourse.bass_utils import run_bass_kernel_spmd

F32 = mybir.dt.float32
BF16 = mybir.dt.bfloat16
AX = mybir.AxisListType
OP = mybir.AluOpType
ACT = mybir.ActivationFunctionType

NCORES = 8
NI = 8                                   # images per core
C0, C1, C2 = 512, 256, 1024
H1, HW1 = 28, 784
H2, HW2 = 14, 196
PIX1, PIX2 = NI * HW1, NI * HW2          # 6272, 1568
K0, KM1, KM2 = C0 // 128, C1 // 128, C2 // 128   # 4, 2, 8
NCH = 392                                # pixel chunk (2 imgs at 14x14 / half img at 28x28)
PW = 29                                  # padded image width (top/left zero pad)
PIMG = PW * PW                           # 841
NG1 = 64 * HW1
NG2 = 64 * HW2
EPS = 1e-5
BF = ml_dtypes.bfloat16

TAPS = [(dy, dx) for dy in range(3) for dx in range(3)]

_CACHE = {}


def _ternarize_host(w):
    """fp32 ternarize matching the jax reference: returns (S, alpha)."""
    w = np.asarray(w, np.float32)
    absw = np.abs(w)
    delta = (0.7 * absw.mean(axis=(1, 2, 3), keepdims=True)).astype(np.float32)
    mask = (absw > delta).astype(np.float32)
    alpha = (absw * mask).sum(axis=(1, 2, 3)) / (mask.sum(axis=(1, 2, 3)) + 1e-8)
    sign = np.sign(w).astype(np.float32)
    return (sign * mask).astype(np.float32), alpha.astype(np.float32)


def build_program():
    nc = bacc.Bacc("TRN2", target_bir_lowering=False, debug=False,
                   num_devices=NCORES)

    x_d = nc.dram_tensor("x", [C0, PIX1], BF16, kind="ExternalInput")
    t1_d = nc.dram_tensor("t1", [C0, C1], BF16, kind="ExternalInput")
    # t2 pre-arranged on host: [k, i_local, t*C1+o]
    t2_d = nc.dram_tensor("t2", [KM1, 128, 9 * C1], BF16, kind="ExternalInput")
    s2s_d = nc.dram_tensor("s2sum", [C1, C1], BF16, kind="ExternalInput")
    t3_d = nc.dram_tensor("t3", [C1, C2], BF16, kind="ExternalInput")
    dsw_d = nc.dram_tensor("dsw", [C0, C2], BF16, kind="ExternalInput")
    # all per-channel params pre-arranged on host as [128, n] column tiles
    # (one contiguous DMA row per partition instead of thousands of 4B
    # descriptors): [a1(2) a2(2) a3(8) bn1g(4) bn2g(2) bn2b(2) bn3g(2)
    # bn3b(2) dsg(8) dsb(8)] = 40 cols
    par_d = nc.dram_tensor("par", [128, 40], F32, kind="ExternalInput")
    out_d = nc.dram_tensor("out", [C2, PIX2], BF16, kind="ExternalOutput")

    with tile.TileContext(nc) as tc:
        _build(nc, tc, x_d, t1_d, t2_d, s2s_d, t3_d, dsw_d, par_d, out_d)

    nc.compile()
    return nc


def _build(nc, tc, x_d, t1_d, t2_d, s2s_d, t3_d, dsw_d, par_d, out_d):
    from contextlib import ExitStack

    gctx = ExitStack()
    with gctx:
        dram = gctx.enter_context(tc.tile_pool(name="dram", bufs=1, space="DRAM"))
        sb = gctx.enter_context(tc.tile_pool(name="sb", bufs=1))
        sb_out = gctx.enter_context(tc.tile_pool(name="sb_out", bufs=4))
        ps_a = gctx.enter_context(tc.tile_pool(name="ps_a", bufs=4, space="PSUM"))
        ps_b = gctx.enter_context(tc.tile_pool(name="ps_b", bufs=4, space="PSUM"))

        # ------------- static loads -------------
        # tiles declared first; DMAs issued in priority order below
        t1s = [sb.tile([128, C1], BF16, name=f"t1_{k}") for k in range(K0)]
        dsws = [sb.tile([128, C2], BF16, name=f"dsw_{k}") for k in range(K0)]
        t2s = [sb.tile([128, 9, C1], BF16, name=f"t2_{k}")
               for k in range(KM1)]
        s2s = [sb.tile([128, C1], BF16, name=f"s2s_{k}") for k in range(KM1)]
        t3s = [sb.tile([128, C2], BF16, name=f"t3_{k}") for k in range(KM1)]
        par = sb.tile([128, 40], F32, name="par")
        a1c = par[:, 0:2]
        a2c = par[:, 2:4]
        a3c = par[:, 4:12]
        bn1g = par[:, 12:16]
        bn2g = par[:, 16:18]
        bn2b = par[:, 18:20]
        bn3g = par[:, 20:22]
        bn3b = par[:, 22:24]
        dsg = par[:, 24:32]
        dsb = par[:, 32:40]

        # persistent activations
        xs = [sb.tile([128, PIX1], BF16, name=f"xs_{k}") for k in range(K0)]
        p1 = [sb.tile([128, NI * PIMG], BF16, name=f"p1_{m}")
              for m in range(KM1)]
        q = [sb.tile([128, PIX2], BF16, name=f"q_{m}") for m in range(KM2)]
        p2 = [sb.tile([128, PIX2], BF16, name=f"p2_{m}") for m in range(KM1)]
        dum = sb.tile([128, PIX2], BF16, name="dum")     # ACT scratch
        dum2 = sb.tile([128, PIX2], BF16, name="dum2")   # Pool scratch
        dumd = sb.tile([128, PIX2], BF16, name="dumd")   # DVE scratch

        # stat tiles
        xbn = sb.tile([128, 40 * 6], F32, name="xbn")
        xagg = sb.tile([128, 3 * 2], F32, name="xagg")
        xsum_c = sb.tile([128, K0 * 4], F32, name="xsum_c")
        xsq_c = sb.tile([128, K0 * 4], F32, name="xsq_c")
        st1 = sb.tile([128, 2 * K0], F32, name="st1")
        g1 = sb.tile([128, 2 * K0], F32, name="g1")
        qsum_c = sb.tile([128, KM2 * 4], F32, name="qsum_c")
        p1s_c = sb.tile([128, KM1 * 16], F32, name="p1s_c")
        p1q_c = sb.tile([128, KM1 * 16], F32, name="p1q_c")
        st2 = sb.tile([128, 20], F32, name="st2")
        g2 = sb.tile([128, 20], F32, name="g2")
        p2s_c = sb.tile([128, KM1 * 4], F32, name="p2s_c")
        p2q_c = sb.tile([128, KM1 * 4], F32, name="p2q_c")
        st3 = sb.tile([128, 4], F32, name="st3")
        g3 = sb.tile([128, 4], F32, name="g3")

        # DMA issue order: x pair 0 first (unblocks stats + ds conv), then
        # the weights the first ops need, then the remaining x pairs, then
        # everything not needed until later phases.
        def load_pair(p):
            for k in range(K0):
                nc.sync.dma_start(
                    xs[k][:, p * 2 * HW1:(p + 1) * 2 * HW1],
                    x_d.ap()[k * 128:(k + 1) * 128,
                             p * 2 * HW1:(p + 1) * 2 * HW1])

        load_pair(0)
        for k in range(K0):
            nc.sync.dma_start(dsws[k][:],
                              dsw_d.ap()[k * 128:(k + 1) * 128, :])
        for p in range(1, 4):
            load_pair(p)
        for k in range(K0):
            nc.sync.dma_start(t1s[k][:], t1_d.ap()[k * 128:(k + 1) * 128, :])
        nc.sync.dma_start(par[:], par_d.ap())
        for k in range(KM1):
            nc.sync.dma_start(
                t2s[k][:].rearrange("p t o -> p (t o)"), t2_d.ap()[k])
            nc.sync.dma_start(s2s[k][:],
                              s2s_d.ap()[k * 128:(k + 1) * 128, :])
            nc.sync.dma_start(t3s[k][:],
                              t3_d.ap()[k * 128:(k + 1) * 128, :])

        # ================= phase A: x stats + ds conv =================
        # x statistics: bn_stats (sum+var in one 1.19ns/elem pass) on DVE
        # for 10 chunks; ACT takes the remaining 6 as Copy-accum (sum) +
        # Square-accum (sumsq). Accumulator ops get no 16-bit speedup, so
        # this split balances the lanes.
        NBN = {0: 4, 1: 4, 2: 2, 3: 0}    # bn_stats chunks per k (pairs 0..)
        # pass 1: ONLY the x statistics (AR1 critical path). The ds conv
        # and its psum drains are emitted after the AR1 trigger so the
        # limited-lookahead scheduler never stalls a stat lane on a psum.
        for p in range(4):
            for k in range(K0):
                chunk = xs[k][:, p * 1568:(p + 1) * 1568]
                c = k * 4 + p
                if p < NBN[k]:
                    base = c * 4
                    for g in range(4):
                        nc.vector.bn_stats(
                            xbn[:, (base + g) * 6:(base + g + 1) * 6],
                            xs[k][:, p * 1568 + g * 392:
                                  p * 1568 + (g + 1) * 392])
                else:
                    nc.scalar.activation(
                        dum[:], chunk, ACT.Copy,
                        accum_out=xsum_c[:, c:c + 1])
                    nc.scalar.activation(
                        dum[:], chunk, ACT.Square,
                        accum_out=xsq_c[:, c:c + 1])

        # x stat reduction -> st1 -> AR1
        # bn side: per k aggregate its bn_stats groups into sum / sumsq
        xbnv = xbn[:].rearrange("p (g s) -> p g s", s=6)
        tmp1 = sb.tile([128, 1], F32, tag="xa", name="xa_tmp1")
        tmp2 = sb.tile([128, 1], F32, tag="xa", name="xa_tmp2")
        for k in range(3):
            ng = NBN[k] * 4
            g0 = (k * 16)
            nc.vector.bn_aggr(xagg[:, k * 2:k * 2 + 2],
                              xbnv[:, g0:g0 + ng, :])
            cnt = float(NBN[k] * 1568)
            mcol = xagg[:, k * 2:k * 2 + 1]
            vcol = xagg[:, k * 2 + 1:k * 2 + 2]
            nc.vector.tensor_scalar(st1[:, k:k + 1], mcol, cnt, None, OP.mult)
            nc.vector.tensor_tensor(tmp1[:], mcol, mcol, OP.mult)
            nc.vector.tensor_tensor(tmp1[:], tmp1[:], vcol, OP.add)
            nc.vector.tensor_scalar(st1[:, K0 + k:K0 + k + 1], tmp1[:],
                                    cnt, None, OP.mult)
        # ACT side: add the accumulated sums for chunks not covered by bn
        nc.vector.tensor_tensor(tmp1[:], xsum_c[:, 10:11], xsum_c[:, 11:12],
                                OP.add)
        nc.vector.tensor_tensor(st1[:, 2:3], st1[:, 2:3], tmp1[:], OP.add)
        nc.vector.tensor_tensor(tmp2[:], xsq_c[:, 10:11], xsq_c[:, 11:12],
                                OP.add)
        nc.vector.tensor_tensor(st1[:, K0 + 2:K0 + 3], st1[:, K0 + 2:K0 + 3],
                                tmp2[:], OP.add)
        nc.vector.reduce_sum(st1[:, 3:4], xsum_c[:, 12:16], axis=AX.X)
        nc.vector.reduce_sum(st1[:, K0 + 3:K0 + 4], xsq_c[:, 12:16],
                             axis=AX.X)
        ar1_in = dram.tile([128, 2 * K0], F32, name="ar1_in")
        ar1_out = dram.tile([128, 2 * K0], F32, name="ar1_out")
        nc.sync.dma_start(ar1_in[:], st1[:])
        nc.gpsimd.collective_compute(
            "AllReduce", OP.add, replica_groups=[list(range(NCORES))],
            ins=[ar1_in.opt()], outs=[ar1_out.opt()])
        nc.sync.dma_start(g1[:], ar1_out[:])

        # pass 2: ds conv + q psum drains + q stats — all overlap AR1
        for p in range(4):
            for m in range(KM2):
                pool = ps_a if m % 2 == 0 else ps_b
                tagn = "mm" if m % 2 == 0 else "c2"
                pmm = pool.tile([128, NCH], F32, tag=tagn,
                                name=f"psds_{p}_{m}")
                for k in range(K0):
                    rhs = xs[k][:, p * 1568:(p + 1) * 1568].rearrange(
                        "p (i ay by ax bx) -> p i by bx ay ax",
                        i=2, ay=14, by=2, ax=14, bx=2)[:, :, 0, 0, :, :]
                    nc.tensor.matmul(
                        pmm[:], dsws[k][:, m * 128:(m + 1) * 128],
                        rhs, start=(k == 0), stop=(k == K0 - 1))
                dst = q[m][:, p * NCH:(p + 1) * NCH]
                acc = qsum_c[:, m * 4 + p:m * 4 + p + 1]
                if m % 2 == 0:
                    nc.vector.tensor_scalar(dst, pmm[:], 1.0, 0.0,
                                            OP.mult, OP.add, accum_out=acc)
                else:
                    nc.scalar.activation(dst, pmm[:], ACT.Copy, accum_out=acc)

        # q sumsq (ACT) + q sum reduce (DVE) — overlap AR1
        for m in range(KM2):
            nc.scalar.activation(dum[:], q[m][:], ACT.Square,
                                 accum_out=st2[:, 12 + m:13 + m])
            nc.vector.reduce_sum(st2[:, 4 + m:5 + m],
                                 qsum_c[:, m * 4:(m + 1) * 4], axis=AX.X)

        # ---- BN1 affine -> fold into t1 ----
        mean_x = sb.tile([128, K0], F32, name="mean_x")
        var_x = sb.tile([128, K0], F32, name="var_x")
        a1f = sb.tile([128, K0], F32, name="a1f")
        tmp_k0 = sb.tile([128, K0], F32, name="tmp_k0")
        nc.vector.tensor_scalar(mean_x[:], g1[:, 0:K0], 1.0 / NG1, None, OP.mult)
        nc.vector.tensor_tensor(tmp_k0[:], mean_x[:], mean_x[:], OP.mult)
        nc.vector.tensor_scalar(var_x[:], g1[:, K0:2 * K0], 1.0 / NG1, None,
                                OP.mult)
        nc.vector.tensor_tensor(var_x[:], var_x[:], tmp_k0[:], OP.subtract)

        def rsqrt_cols(dst, var_ap, gamma_ap, extra_mul=None):
            cols = dst.shape[1]
            tmp = sb.tile([128, cols], F32, tag="rsq_tmp",
                          name=f"rsq_{dst.tensor.name}")
            nc.vector.tensor_scalar(tmp[:], var_ap, EPS, None, OP.add)
            nc.vector.reciprocal(tmp[:], tmp[:])
            nc.scalar.sqrt(tmp[:], tmp[:])
            nc.vector.tensor_tensor(dst, tmp[:], gamma_ap, OP.mult)
            if extra_mul is not None:
                nc.vector.tensor_tensor(dst, dst, extra_mul, OP.mult)

        rsqrt_cols(a1f[:], var_x[:], bn1g)
        for k in range(K0):
            nc.vector.tensor_scalar(t1s[k][:], t1s[k][:], a1f[:, k:k + 1],
                                    None, OP.mult)

        # ================= conv1 (raw x @ folded t1) =================
        # psum->p1 copies alternate DVE/ACT (both with sum accumulators);
        # sumsq per chunk on Pool (scalar_tensor_tensor square).
        for m in range(KM1):
            for img in range(NI):
                for half in range(2):
                    pmm = ps_a.tile([128, NCH], F32, tag="mm",
                                    name=f"psc1_{m}_{img}_{half}")
                    for k in range(K0):
                        nc.tensor.matmul(
                            pmm[:], t1s[k][:, m * 128:(m + 1) * 128],
                            xs[k][:, img * HW1 + half * NCH:
                                  img * HW1 + (half + 1) * NCH],
                            start=(k == 0), stop=(k == K0 - 1))
                    dst = p1[m][:, img * PIMG:(img + 1) * PIMG].rearrange(
                        "p (r c) -> p r c", r=PW, c=PW)[
                        :, 1 + half * 14:1 + (half + 1) * 14, 1:PW]
                    src = pmm[:].rearrange("p (r c) -> p r c", r=14, c=28)
                    slot = m * 16 + img * 2 + half
                    sacc = p1s_c[:, slot:slot + 1]
                    qacc = p1q_c[:, slot:slot + 1]
                    if (img + half) % 2 == 0:
                        nc.vector.tensor_scalar(dst, src, 1.0, 0.0,
                                                OP.mult, OP.add,
                                                accum_out=sacc)
                        nc.scalar.activation(dum[:, 0:NCH], pmm[:],
                                             ACT.Square, accum_out=qacc)
                    else:
                        nc.scalar.activation(dst, src, ACT.Copy,
                                             accum_out=sacc)
                        nc.vector.scalar_tensor_tensor(
                            dumd[:, 0:NCH].rearrange(
                                "p (r c) -> p r c", r=14, c=28),
                            dst, 1.0, dst, OP.mult, OP.mult,
                            accum_out=qacc)

        # p1 stat reduction -> st2 -> AR2
        p1sv = p1s_c[:].rearrange("p (m c) -> p m c", c=16)
        p1qv = p1q_c[:].rearrange("p (m c) -> p m c", c=16)
        for m in range(KM1):
            nc.vector.reduce_sum(st2[:, m:m + 1], p1sv[:, m], axis=AX.X)
            nc.vector.reduce_sum(st2[:, KM1 + m:KM1 + m + 1], p1qv[:, m],
                                 axis=AX.X)
        ar2_in = dram.tile([128, 20], F32, name="ar2_in")
        ar2_out = dram.tile([128, 20], F32, name="ar2_out")
        nc.sync.dma_start(ar2_in[:], st2[:])
        nc.gpsimd.collective_compute(
            "AllReduce", OP.add, replica_groups=[list(range(NCORES))],
            ins=[ar2_in.opt()], outs=[ar2_out.opt()])
        nc.sync.dma_start(g2[:], ar2_out[:])

        # ---- post-AR2 column math ----
        mean_p1 = sb.tile([128, KM1], F32, name="mean_p1")
        var_p1 = sb.tile([128, KM1], F32, name="var_p1")
        a2f = sb.tile([128, KM1], F32, name="a2f")
        b2f = sb.tile([128, KM1], F32, name="b2f")
        v2 = sb.tile([128, KM1], F32, name="v2")
        b2fb = sb.tile([128, KM1], BF16, name="b2fb")
        tmp_m1 = sb.tile([128, KM1], F32, name="tmp_m1")
        nc.vector.tensor_scalar(mean_p1[:], g2[:, 0:2], 1.0 / NG1, None, OP.mult)
        nc.vector.tensor_scalar(var_p1[:], g2[:, 2:4], 1.0 / NG1, None, OP.mult)
        nc.vector.tensor_tensor(tmp_m1[:], mean_p1[:], mean_p1[:], OP.mult)
        nc.vector.tensor_tensor(var_p1[:], var_p1[:], tmp_m1[:], OP.subtract)
        nc.vector.tensor_tensor(tmp_m1[:], a1c, a1c, OP.mult)
        nc.vector.tensor_tensor(var_p1[:], var_p1[:], tmp_m1[:], OP.mult)
        rsqrt_cols(a2f[:], var_p1[:], bn2g, extra_mul=a1c)

        # fold a2f into t2 weights; fill p1 pads with v2 — these unblock
        # the conv2 matmuls, so they come before the rest of the col math
        for k in range(KM1):
            nc.vector.tensor_scalar(
                t2s[k][:].rearrange("p t o -> p (t o)"),
                t2s[k][:].rearrange("p t o -> p (t o)"),
                a2f[:, k:k + 1], None, OP.mult)
        # v2 = -b2f/a2f = mean_p1 - bn2b/a2f
        nc.vector.reciprocal(tmp_m1[:], a2f[:])
        nc.vector.tensor_tensor(tmp_m1[:], bn2b, tmp_m1[:], OP.mult)
        nc.vector.tensor_tensor(v2[:], mean_p1[:], tmp_m1[:], OP.subtract)
        for m in range(KM1):
            pv = p1[m][:].rearrange("p (i r c) -> p i r c", i=NI, r=PW, c=PW)
            nc.vector.tensor_copy(
                pv[:, :, 0, :], v2[:, m:m + 1].broadcast_to([128, NI, PW]))
            nc.vector.tensor_copy(
                pv[:, :, 1:PW, 0], v2[:, m:m + 1].broadcast_to([128, NI, 28]))

        nc.vector.tensor_tensor(tmp_m1[:], a2f[:], mean_p1[:], OP.mult)
        nc.vector.tensor_tensor(b2f[:], bn2b, tmp_m1[:], OP.subtract)
        nc.vector.tensor_copy(b2fb[:], b2f[:])

        # ds BN affine (uses AR2 q stats; needed only for the q affine)
        mean_q = sb.tile([128, KM2], F32, name="mean_q")
        var_q = sb.tile([128, KM2], F32, name="var_q")
        aq = sb.tile([128, KM2], F32, name="aq")
        bq = sb.tile([128, KM2], F32, name="bq")
        tmp_m2 = sb.tile([128, KM2], F32, name="tmp_m2")
        nc.vector.tensor_scalar(mean_q[:], g2[:, 4:12], 1.0 / NG2, None, OP.mult)
        nc.vector.tensor_scalar(var_q[:], g2[:, 12:20], 1.0 / NG2, None, OP.mult)
        nc.vector.tensor_tensor(tmp_m2[:], mean_q[:], mean_q[:], OP.mult)
        nc.vector.tensor_tensor(var_q[:], var_q[:], tmp_m2[:], OP.subtract)
        rsqrt_cols(aq[:], var_q[:], dsg)
        nc.vector.tensor_tensor(tmp_m2[:], aq[:], mean_q[:], OP.mult)
        nc.vector.tensor_tensor(bq[:], dsb, tmp_m2[:], OP.subtract)

        # T[o] = sum_i S2sum[i,o] * b2f[i]  (border-free BN2 shift)
        Tc = sb.tile([128, KM1], F32, name="Tc")
        for m in range(KM1):
            tps = ps_b.tile([128, NCH], F32, tag="c2", name=f"tps_{m}")
            for k in range(KM1):
                nc.tensor.matmul(tps[:, 0:1], s2s[k][:, m * 128:(m + 1) * 128],
                                 b2fb[:, k:k + 1],
                                 start=(k == 0), stop=(k == KM1 - 1))
            nc.vector.tensor_copy(Tc[:, m:m + 1], tps[:, 0:1])

        # ================= conv2: 3x3 s2 p1 from padded p1 =================
        p1v = [p1[k][:].rearrange("p (i r c) -> p i r c", i=NI, r=PW, c=PW)
               for k in range(KM1)]
        for m in range(KM1):
            pmms = [ps_b.tile([128, NCH], F32, tag="c2",
                              name=f"psc2_{m}_{ip}") for ip in range(4)]
            for t, (dy, dx) in enumerate(TAPS):
                for k in range(KM1):
                    for ip in range(4):
                        rhs = p1v[k][:, 2 * ip:2 * ip + 2,
                                     dy:dy + 27:2, dx:dx + 27:2]
                        nc.tensor.matmul(
                            pmms[ip][:],
                            t2s[k][:, t, m * 128:(m + 1) * 128],
                            rhs, start=(t == 0 and k == 0),
                            stop=(t == 8 and k == KM1 - 1))
            for ip in range(4):
                slot = m * 4 + ip
                nc.vector.tensor_scalar(
                    p2[m][:, ip * NCH:(ip + 1) * NCH], pmms[ip][:],
                    1.0, Tc[:, m:m + 1], OP.mult, OP.add,
                    accum_out=p2s_c[:, slot:slot + 1])
                nc.scalar.activation(
                    dum[:, 0:NCH], p2[m][:, ip * NCH:(ip + 1) * NCH],
                    ACT.Square, accum_out=p2q_c[:, slot:slot + 1])

        # p2 stats -> st3 -> AR3
        for m in range(KM1):
            nc.vector.reduce_sum(st3[:, m:m + 1],
                                 p2s_c[:, m * 4:(m + 1) * 4], axis=AX.X)
            nc.vector.reduce_sum(st3[:, 2 + m:3 + m],
                                 p2q_c[:, m * 4:(m + 1) * 4], axis=AX.X)
        ar3_in = dram.tile([128, 4], F32, name="ar3_in")
        ar3_out = dram.tile([128, 4], F32, name="ar3_out")
        nc.sync.dma_start(ar3_in[:], st3[:])
        nc.gpsimd.collective_compute(
            "AllReduce", OP.add, replica_groups=[list(range(NCORES))],
            ins=[ar3_in.opt()], outs=[ar3_out.opt()])
        nc.sync.dma_start(g3[:], ar3_out[:])

        # shortcut affine in place on q — overlaps AR3 (needs only AR2 stats)
        for m in range(KM2):
            nc.vector.tensor_scalar(q[m][:], q[m][:], aq[:, m:m + 1],
                                    bq[:, m:m + 1], OP.mult, OP.add)

        # ---- BN3 affine (alpha2-corrected) -> xn3 in place on p2 ----
        mean_p2 = sb.tile([128, KM1], F32, name="mean_p2")
        var_p2 = sb.tile([128, KM1], F32, name="var_p2")
        a3f = sb.tile([128, KM1], F32, name="a3f")
        b3f = sb.tile([128, KM1], F32, name="b3f")
        tmp_m3 = sb.tile([128, KM1], F32, name="tmp_m3")
        nc.vector.tensor_scalar(mean_p2[:], g3[:, 0:2], 1.0 / NG2, None, OP.mult)
        nc.vector.tensor_scalar(var_p2[:], g3[:, 2:4], 1.0 / NG2, None, OP.mult)
        nc.vector.tensor_tensor(tmp_m3[:], mean_p2[:], mean_p2[:], OP.mult)
        nc.vector.tensor_tensor(var_p2[:], var_p2[:], tmp_m3[:], OP.subtract)
        nc.vector.tensor_tensor(tmp_m3[:], a2c, a2c, OP.mult)
        nc.vector.tensor_tensor(var_p2[:], var_p2[:], tmp_m3[:], OP.mult)
        rsqrt_cols(a3f[:], var_p2[:], bn3g, extra_mul=a2c)
        nc.vector.tensor_tensor(tmp_m3[:], a3f[:], mean_p2[:], OP.mult)
        nc.vector.tensor_tensor(b3f[:], bn3b, tmp_m3[:], OP.subtract)
        for m in range(KM1):
            nc.vector.tensor_scalar(p2[m][:], p2[m][:], a3f[:, m:m + 1],
                                    b3f[:, m:m + 1], OP.mult, OP.add)

        # ================= conv3 + residual fuse + store =================
        for m in range(KM2):
            for j in range(4):
                pmm = ps_a.tile([128, NCH], F32, tag="mm",
                                name=f"psc3_{m}_{j}")
                for k in range(KM1):
                    nc.tensor.matmul(
                        pmm[:], t3s[k][:, m * 128:(m + 1) * 128],
                        p2[k][:, j * NCH:(j + 1) * NCH],
                        start=(k == 0), stop=(k == KM1 - 1))
                out_t = sb_out.tile([128, NCH], BF16, tag="out",
                                    name=f"out_{m}_{j}")
                qc = q[m][:, j * NCH:(j + 1) * NCH]
                if j == 0:
                    nc.vector.scalar_tensor_tensor(
                        out_t[:], pmm[:], a3c[:, m:m + 1], qc,
                        OP.mult, OP.add)
                else:
                    # ACT scales out of psum, DVE adds the shortcut (2x)
                    nc.scalar.activation(out_t[:], pmm[:], ACT.Copy,
                                         scale=a3c[:, m:m + 1])
                    nc.vector.tensor_tensor(out_t[:], out_t[:], qc, OP.add)
                nc.sync.dma_start(
                    out_d.ap()[m * 128:(m + 1) * 128,
                               j * NCH:(j + 1) * NCH],
                    out_t[:])


def _prep_host(inputs):
    """Host-side: shard x, fold weights, build per-core in_maps."""
    x = np.asarray(inputs["x"], np.float32)          # [64, 512, 28, 28]
    w1 = np.asarray(inputs["w1"], np.float32)
    w2 = np.asarray(inputs["w2"], np.float32)
    w3 = np.asarray(inputs["w3"], np.float32)
    ds_w = np.asarray(inputs["ds_w"], np.float32)

    s1, al1 = _ternarize_host(w1)    # [256,512,1,1]
    s2, al2 = _ternarize_host(w2)    # [256,256,3,3]
    s3, al3 = _ternarize_host(w3)    # [1024,256,1,1]

    t1 = np.ascontiguousarray(s1[:, :, 0, 0].T).astype(BF)       # [512, 256]
    # t2[t, i, o] = s2[o, i, ky, kx], t = ky*3+kx; shipped pre-tiled as
    # [k, i_local, t*C1 + o] so each partition row is one contiguous DMA
    t2 = s2.transpose(2, 3, 1, 0).reshape(9, C1, C1)
    s2sum = t2.sum(axis=0).astype(BF)                            # [256, 256]
    t2k = np.ascontiguousarray(
        t2.transpose(1, 0, 2).reshape(KM1, 128, 9 * C1)).astype(BF)
    t3 = np.ascontiguousarray(s3[:, :, 0, 0].T).astype(BF)       # [256, 1024]
    dsw = np.ascontiguousarray(ds_w[:, :, 0, 0].T).astype(BF)    # [512, 1024]

    def cols(v, n):
        # [n*128] channel vector -> [128, n] column layout
        return np.asarray(v, np.float32).reshape(n, 128).T

    par = np.concatenate([
        cols(al1, 2), cols(al2, 2), cols(al3, 8),
        cols(inputs["bn1_g"], 4), cols(inputs["bn2_g"], 2),
        cols(inputs["bn2_b"], 2), cols(inputs["bn3_g"], 2),
        cols(inputs["bn3_b"], 2), cols(inputs["ds_bn_g"], 8),
        cols(inputs["ds_bn_b"], 8),
    ], axis=1)
    par = np.ascontiguousarray(par, dtype=np.float32)            # [128, 40]

    common = dict(t1=t1, t2=t2k, s2sum=s2sum, t3=t3, dsw=dsw, par=par)

    in_maps = []
    for c in range(NCORES):
        xc = x[c * NI:(c + 1) * NI]                      # [8, 512, 28, 28]
        xc = np.ascontiguousarray(
            xc.transpose(1, 0, 2, 3).reshape(C0, PIX1)).astype(BF)
        in_maps.append({"x": xc, **common})
    return in_maps


def kernel(**inputs):
    if "nc" not in _CACHE:
        _CACHE["nc"] = build_program()
    nc = _CACHE["nc"]

    in_maps = _prep_host(inputs)
    try:
        res = run_bass_kernel_spmd(nc, in_maps, core_ids=list(range(NCORES)))
    except Exception:
        # transient device state (e.g. a previous crashed run) usually
        # clears on retry
        res = run_bass_kernel_spmd(nc, in_maps, core_ids=list(range(NCORES)))

    out = np.empty((64, C2, H2, H2), np.float32)
    for c in range(NCORES):
        oc = np.asarray(res.results[c]["out"]).astype(np.float32)
        oc = oc.reshape(C2, NI, H2, H2)
        out[c * NI:(c + 1) * NI] = oc.transpose(1, 0, 2, 3)
    return out


# revision 50
# speedup vs baseline: 1.0085x; 1.0085x over previous
"""Trainium2 Bass kernel for nn_BottleneckR (pre-activation ternary bottleneck).

Reference (batch 64):
  h  = conv1x1(BN1(x), tern(w1))            [64,256,28,28]
  h  = conv3x3s2p1(BN2(h), tern(w2))        [64,256,14,14]
  h  = conv1x1(BN3(h), tern(w3))            [64,1024,14,14]
  sc = BN_ds(conv1x1s2(x, ds_w))            [64,1024,14,14]
  out = h + sc

Strategy: data-parallel over batch on 8 NeuronCores (8 images/core).
Exact sync-BN via 3 small AllReduces of per-channel sum/sumsq.

Differences vs the v1 kernel (306us):
 - bf16 end to end: x is converted to bf16 on the host (halves the HBM
   stream), all matmuls run bf16 at full PE rate, DVE elementwise ops get
   the 2x 16-bit mode. Ternary sign matrices are exact in bf16.
 - x stays resident in SBUF (one HBM read instead of two).
 - p1 is stored in a 29x29 zero-padded per-image layout; conv2's 9 taps
   read it directly with stride-2 access patterns (no phase-split scatter,
   no shifted-plane copies).
 - BN2 is folded into conv2 instead of materialized: the scale a2f folds
   into the t2 weights (per input channel), the shift b2f is handled by
   adding T[o] = sum_i S2sum[i,o]*b2f[i] at the psum->p2 copy while the
   pad cells hold -b2f/a2f (so border taps contribute zero shift, exactly
   matching zero padding after BN).
 - AR1 is triggered as soon as the x statistics are done; the ds-conv
   tail, q sum-of-squares and all q/p1 copies overlap the collective.
 - q's BN affine runs during AR3's latency window.
 - One persistent tile pool: no mid-kernel pool-exit barriers (the v1
   kernel lost ~16us of DVE time to one of those after AR3).
"""

import sys

sys.path.insert(0, "/opt/trn_rl_repo")

import numpy as np
import ml_dtypes

import concourse.bacc as bacc
import concourse.mybir as mybir
import concourse.tile as tile
from concourse.bass_utils import run_bass_kernel_spmd

F32 = mybir.dt.float32
BF16 = mybir.dt.bfloat16
AX = mybir.AxisListType
OP = mybir.AluOpType
ACT = mybir.ActivationFunctionType

NCORES = 8
NI = 8                                   # images per core
C0, C1, C2 = 512, 256, 1024
H1, HW1 = 28, 784
H2, HW2 = 14, 196
PIX1, PIX2 = NI * HW1, NI * HW2          # 6272, 1568
K0, KM1, KM2 = C0 // 128, C1 // 128, C2 // 128   # 4, 2, 8
NCH = 392                                # pixel chunk (2 imgs at 14x14 / half img at 28x28)
PW = 29                                  # padded image width (top/left zero pad)
PIMG = PW * PW                           # 841
NG1 = 64 * HW1
NG2 = 64 * HW2
EPS = 1e-5
BF = ml_dtypes.bfloat16

TAPS = [(dy, dx) for dy in range(3) for dx in range(3)]

_CACHE = {}


def _ternarize_host(w):
    """fp32 ternarize matching the jax reference: returns (S, alpha)."""
    w = np.asarray(w, np.float32)
    absw = np.abs(w)
    delta = (0.7 * absw.mean(axis=(1, 2, 3), keepdims=True)).astype(np.float32)
    mask = (absw > delta).astype(np.float32)
    alpha = (absw * mask).sum(axis=(1, 2, 3)) / (mask.sum(axis=(1, 2, 3)) + 1e-8)
    sign = np.sign(w).astype(np.float32)
    return (sign * mask).astype(np.float32), alpha.astype(np.float32)


def build_program():
    nc = bacc.Bacc("TRN2", target_bir_lowering=False, debug=False,
                   num_devices=NCORES)

    x_d = nc.dram_tensor("x", [C0, PIX1], BF16, kind="ExternalInput")
    t1_d = nc.dram_tensor("t1", [C0, C1], BF16, kind="ExternalInput")
    # t2 pre-arranged on host: [k, i_local, t*C1+o]
    t2_d = nc.dram_tensor("t2", [KM1, 128, 9 * C1], BF16, kind="ExternalInput")
    s2s_d = nc.dram_tensor("s2sum", [C1, C1], BF16, kind="ExternalInput")
    t3_d = nc.dram_tensor("t3", [C1, C2], BF16, kind="ExternalInput")
    dsw_d = nc.dram_tensor("dsw", [C0, C2], BF16, kind="ExternalInput")
    # all per-channel params pre-arranged on host as [128, n] column tiles
    # (one contiguous DMA row per partition instead of thousands of 4B
    # descriptors): [a1(2) a2(2) a3(8) bn1g(4) bn2g(2) bn2b(2) bn3g(2)
    # bn3b(2) dsg(8) dsb(8)] = 40 cols
    par_d = nc.dram_tensor("par", [128, 40], F32, kind="ExternalInput")
    out_d = nc.dram_tensor("out", [C2, PIX2], BF16, kind="ExternalOutput")

    with tile.TileContext(nc) as tc:
        _build(nc, tc, x_d, t1_d, t2_d, s2s_d, t3_d, dsw_d, par_d, out_d)

    nc.compile()
    return nc


def _build(nc, tc, x_d, t1_d, t2_d, s2s_d, t3_d, dsw_d, par_d, out_d):
    from contextlib import ExitStack

    gctx = ExitStack()
    with gctx:
        dram = gctx.enter_context(tc.tile_pool(name="dram", bufs=1, space="DRAM"))
        sb = gctx.enter_context(tc.tile_pool(name="sb", bufs=1))
        sb_out = gctx.enter_context(tc.tile_pool(name="sb_out", bufs=4))
        ps_a = gctx.enter_context(tc.tile_pool(name="ps_a", bufs=4, space="PSUM"))
        ps_b = gctx.enter_context(tc.tile_pool(name="ps_b", bufs=4, space="PSUM"))

        # ------------- static loads -------------
        # tiles declared first; DMAs issued in priority order below
        t1s = [sb.tile([128, C1], BF16, name=f"t1_{k}") for k in range(K0)]
        dsws = [sb.tile([128, C2], BF16, name=f"dsw_{k}") for k in range(K0)]
        t2s = [sb.tile([128, 9, C1], BF16, name=f"t2_{k}")
               for k in range(KM1)]
        s2s = [sb.tile([128, C1], BF16, name=f"s2s_{k}") for k in range(KM1)]
        t3s = [sb.tile([128, C2], BF16, name=f"t3_{k}") for k in range(KM1)]
        par = sb.tile([128, 40], F32, name="par")
        a1c = par[:, 0:2]
        a2c = par[:, 2:4]
        a3c = par[:, 4:12]
        bn1g = par[:, 12:16]
        bn2g = par[:, 16:18]
        bn2b = par[:, 18:20]
        bn3g = par[:, 20:22]
        bn3b = par[:, 22:24]
        dsg = par[:, 24:32]
        dsb = par[:, 32:40]

        # persistent activations
        xs = [sb.tile([128, PIX1], BF16, name=f"xs_{k}") for k in range(K0)]
        p1 = [sb.tile([128, NI * PIMG], BF16, name=f"p1_{m}")
              for m in range(KM1)]
        q = [sb.tile([128, PIX2], BF16, name=f"q_{m}") for m in range(KM2)]
        p2 = [sb.tile([128, PIX2], BF16, name=f"p2_{m}") for m in range(KM1)]
        dum = sb.tile([128, PIX2], BF16, name="dum")     # ACT scratch
        dum2 = sb.tile([128, PIX2], BF16, name="dum2")   # Pool scratch
        dumd = sb.tile([128, PIX2], BF16, name="dumd")   # DVE scratch

        # stat tiles
        xbn = sb.tile([128, 48 * 6], F32, name="xbn")
        xagg = sb.tile([128, 3 * 2], F32, name="xagg")
        xsum_c = sb.tile([128, K0 * 4], F32, name="xsum_c")
        xsq_c = sb.tile([128, K0 * 4], F32, name="xsq_c")
        st1 = sb.tile([128, 2 * K0], F32, name="st1")
        g1 = sb.tile([128, 2 * K0], F32, name="g1")
        qsum_c = sb.tile([128, KM2 * 4], F32, name="qsum_c")
        p1bn = sb.tile([128, KM1 * 16 * 6], F32, name="p1bn")
        p1agg = sb.tile([128, KM1 * 2], F32, name="p1agg")
        st2 = sb.tile([128, 20], F32, name="st2")
        g2 = sb.tile([128, 20], F32, name="g2")
        p2bn = sb.tile([128, KM1 * 4 * 6], F32, name="p2bn")
        p2agg = sb.tile([128, KM1 * 2], F32, name="p2agg")
        st3 = sb.tile([128, 4], F32, name="st3")
        g3 = sb.tile([128, 4], F32, name="g3")

        # DMA issue order: x pair 0 first (unblocks stats + ds conv), then
        # the weights the first ops need, then the remaining x pairs, then
        # everything not needed until later phases.
        def load_pair(p):
            for k in range(K0):
                nc.sync.dma_start(
                    xs[k][:, p * 2 * HW1:(p + 1) * 2 * HW1],
                    x_d.ap()[k * 128:(k + 1) * 128,
                             p * 2 * HW1:(p + 1) * 2 * HW1])

        load_pair(0)
        for k in range(K0):
            nc.sync.dma_start(dsws[k][:],
                              dsw_d.ap()[k * 128:(k + 1) * 128, :])
        for p in range(1, 4):
            load_pair(p)
        for k in range(K0):
            nc.sync.dma_start(t1s[k][:], t1_d.ap()[k * 128:(k + 1) * 128, :])
        nc.sync.dma_start(par[:], par_d.ap())
        for k in range(KM1):
            nc.sync.dma_start(
                t2s[k][:].rearrange("p t o -> p (t o)"), t2_d.ap()[k])
            nc.sync.dma_start(s2s[k][:],
                              s2s_d.ap()[k * 128:(k + 1) * 128, :])
            nc.sync.dma_start(t3s[k][:],
                              t3_d.ap()[k * 128:(k + 1) * 128, :])

        # ================= phase A: x stats + ds conv =================
        # x statistics: bn_stats (sum+var in one 1.19ns/elem pass) on DVE
        # for 10 chunks; ACT takes the remaining 6 as Copy-accum (sum) +
        # Square-accum (sumsq). Accumulator ops get no 16-bit speedup, so
        # this split balances the lanes.
        NBN = {0: 4, 1: 4, 2: 4, 3: 0}    # bn_stats chunks per k (pairs 0..)
        # pass 1: ONLY the x statistics (AR1 critical path). The ds conv
        # and its psum drains are emitted after the AR1 trigger so the
        # limited-lookahead scheduler never stalls a stat lane on a psum.
        for p in range(4):
            for k in range(K0):
                chunk = xs[k][:, p * 1568:(p + 1) * 1568]
                c = k * 4 + p
                if p < NBN[k]:
                    base = c * 4
                    for g in range(4):
                        nc.vector.bn_stats(
                            xbn[:, (base + g) * 6:(base + g + 1) * 6],
                            xs[k][:, p * 1568 + g * 392:
                                  p * 1568 + (g + 1) * 392])
                else:
                    nc.scalar.activation(
                        dum[:], chunk, ACT.Copy,
                        accum_out=xsum_c[:, c:c + 1])
                    nc.scalar.activation(
                        dum[:], chunk, ACT.Square,
                        accum_out=xsq_c[:, c:c + 1])

        # x stat reduction -> st1 -> AR1
        # bn side: per k aggregate its bn_stats groups into sum / sumsq
        xbnv = xbn[:].rearrange("p (g s) -> p g s", s=6)
        tmp1 = sb.tile([128, 1], F32, tag="xa", name="xa_tmp1")
        tmp2 = sb.tile([128, 1], F32, tag="xa", name="xa_tmp2")
        for k in range(3):
            ng = NBN[k] * 4
            g0 = (k * 16)
            nc.vector.bn_aggr(xagg[:, k * 2:k * 2 + 2],
                              xbnv[:, g0:g0 + ng, :])
            cnt = float(NBN[k] * 1568)
            mcol = xagg[:, k * 2:k * 2 + 1]
            vcol = xagg[:, k * 2 + 1:k * 2 + 2]
            nc.vector.tensor_scalar(st1[:, k:k + 1], mcol, cnt, None, OP.mult)
            nc.vector.tensor_tensor(tmp1[:], mcol, mcol, OP.mult)
            nc.vector.tensor_tensor(tmp1[:], tmp1[:], vcol, OP.add)
            nc.vector.tensor_scalar(st1[:, K0 + k:K0 + k + 1], tmp1[:],
                                    cnt, None, OP.mult)
        # ACT side: k3's chunks come from the accumulator columns
        nc.vector.reduce_sum(st1[:, 3:4], xsum_c[:, 12:16], axis=AX.X)
        nc.vector.reduce_sum(st1[:, K0 + 3:K0 + 4], xsq_c[:, 12:16],
                             axis=AX.X)
        ar1_in = dram.tile([128, 2 * K0], F32, name="ar1_in")
        ar1_out = dram.tile([128, 2 * K0], F32, name="ar1_out")
        nc.sync.dma_start(ar1_in[:], st1[:])
        nc.gpsimd.collective_compute(
            "AllReduce", OP.add, replica_groups=[list(range(NCORES))],
            ins=[ar1_in.opt()], outs=[ar1_out.opt()])
        nc.sync.dma_start(g1[:], ar1_out[:])

        # pass 2: ds conv + q psum drains + q stats — all overlap AR1
        for p in range(4):
            for m in range(KM2):
                pool = ps_a if m % 2 == 0 else ps_b
                tagn = "mm" if m % 2 == 0 else "c2"
                pmm = pool.tile([128, NCH], F32, tag=tagn,
                                name=f"psds_{p}_{m}")
                for k in range(K0):
                    rhs = xs[k][:, p * 1568:(p + 1) * 1568].rearrange(
                        "p (i ay by ax bx) -> p i by bx ay ax",
                        i=2, ay=14, by=2, ax=14, bx=2)[:, :, 0, 0, :, :]
                    nc.tensor.matmul(
                        pmm[:], dsws[k][:, m * 128:(m + 1) * 128],
                        rhs, start=(k == 0), stop=(k == K0 - 1))
                dst = q[m][:, p * NCH:(p + 1) * NCH]
                acc = qsum_c[:, m * 4 + p:m * 4 + p + 1]
                if m % 2 == 0:
                    nc.vector.tensor_scalar(dst, pmm[:], 1.0, 0.0,
                                            OP.mult, OP.add, accum_out=acc)
                else:
                    nc.scalar.activation(dst, pmm[:], ACT.Copy, accum_out=acc)

        # q sumsq (split DVE/ACT) + q sum reduce — overlap AR1
        for m in range(KM2):
            if m < 4:
                nc.vector.scalar_tensor_tensor(
                    dumd[:], q[m][:], 1.0, q[m][:], OP.mult, OP.mult,
                    accum_out=st2[:, 12 + m:13 + m])
            else:
                nc.scalar.activation(dum[:], q[m][:], ACT.Square,
                                     accum_out=st2[:, 12 + m:13 + m])
            nc.vector.reduce_sum(st2[:, 4 + m:5 + m],
                                 qsum_c[:, m * 4:(m + 1) * 4], axis=AX.X)

        # ---- BN1 affine -> fold into t1 ----
        mean_x = sb.tile([128, K0], F32, name="mean_x")
        var_x = sb.tile([128, K0], F32, name="var_x")
        a1f = sb.tile([128, K0], F32, name="a1f")
        tmp_k0 = sb.tile([128, K0], F32, name="tmp_k0")
        nc.vector.tensor_scalar(mean_x[:], g1[:, 0:K0], 1.0 / NG1, None, OP.mult)
        nc.vector.tensor_tensor(tmp_k0[:], mean_x[:], mean_x[:], OP.mult)
        nc.vector.tensor_scalar(var_x[:], g1[:, K0:2 * K0], 1.0 / NG1, None,
                                OP.mult)
        nc.vector.tensor_tensor(var_x[:], var_x[:], tmp_k0[:], OP.subtract)

        def rsqrt_cols(dst, var_ap, gamma_ap, extra_mul=None):
            cols = dst.shape[1]
            tmp = sb.tile([128, cols], F32, tag="rsq_tmp",
                          name=f"rsq_{dst.tensor.name}")
            nc.vector.tensor_scalar(tmp[:], var_ap, EPS, None, OP.add)
            nc.vector.reciprocal(tmp[:], tmp[:])
            nc.scalar.sqrt(tmp[:], tmp[:])
            nc.vector.tensor_tensor(dst, tmp[:], gamma_ap, OP.mult)
            if extra_mul is not None:
                nc.vector.tensor_tensor(dst, dst, extra_mul, OP.mult)

        rsqrt_cols(a1f[:], var_x[:], bn1g)
        for k in range(K0):
            nc.vector.tensor_scalar(t1s[k][:], t1s[k][:], a1f[:, k:k + 1],
                                    None, OP.mult)

        # ================= conv1 (raw x @ folded t1) =================
        # psum->p1 copies alternate DVE/ACT (both with sum accumulators);
        # sumsq per chunk on Pool (scalar_tensor_tensor square).
        for m in range(KM1):
            for img in range(NI):
                for half in range(2):
                    pmm = ps_a.tile([128, NCH], F32, tag="mm",
                                    name=f"psc1_{m}_{img}_{half}")
                    for k in range(K0):
                        nc.tensor.matmul(
                            pmm[:], t1s[k][:, m * 128:(m + 1) * 128],
                            xs[k][:, img * HW1 + half * NCH:
                                  img * HW1 + (half + 1) * NCH],
                            start=(k == 0), stop=(k == K0 - 1))
                    dst = p1[m][:, img * PIMG:(img + 1) * PIMG].rearrange(
                        "p (r c) -> p r c", r=PW, c=PW)[
                        :, 1 + half * 14:1 + (half + 1) * 14, 1:PW]
                    src = pmm[:].rearrange("p (r c) -> p r c", r=14, c=28)
                    slot = m * 16 + img * 2 + half
                    # both moments in one DVE pass over the psum
                    nc.vector.bn_stats(p1bn[:, slot * 6:(slot + 1) * 6],
                                       pmm[:])
                    if slot % 4 == 0:
                        nc.vector.tensor_copy(dst, src)
                    else:
                        nc.scalar.activation(dst, src, ACT.Copy)

        # p1 stat aggregation -> st2 -> AR2
        for m in range(KM1):
            nc.vector.bn_aggr(
                p1agg[:, m * 2:m * 2 + 2],
                p1bn[:, m * 96:(m + 1) * 96].rearrange("p (g s) -> p g s",
                                                       s=6))
        pav = p1agg[:].rearrange("p (m two) -> p m two", two=2)
        pmsq = sb.tile([128, KM1], F32, name="pmsq")
        nc.vector.tensor_tensor(pmsq[:], pav[:, :, 0], pav[:, :, 0], OP.mult)
        nc.vector.tensor_scalar(st2[:, 0:KM1], pav[:, :, 0], float(PIX1),
                                None, OP.mult)
        nc.vector.tensor_tensor(pmsq[:], pmsq[:], pav[:, :, 1], OP.add)
        nc.vector.tensor_scalar(st2[:, KM1:2 * KM1], pmsq[:], float(PIX1),
                                None, OP.mult)
        ar2_in = dram.tile([128, 20], F32, name="ar2_in")
        ar2_out = dram.tile([128, 20], F32, name="ar2_out")
        nc.sync.dma_start(ar2_in[:], st2[:])
        nc.gpsimd.collective_compute(
            "AllReduce", OP.add, replica_groups=[list(range(NCORES))],
            ins=[ar2_in.opt()], outs=[ar2_out.opt()])
        nc.sync.dma_start(g2[:], ar2_out[:])

        # ---- post-AR2 column math ----
        mean_p1 = sb.tile([128, KM1], F32, name="mean_p1")
        var_p1 = sb.tile([128, KM1], F32, name="var_p1")
        a2f = sb.tile([128, KM1], F32, name="a2f")
        b2f = sb.tile([128, KM1], F32, name="b2f")
        v2 = sb.tile([128, KM1], F32, name="v2")
        b2fb = sb.tile([128, KM1], BF16, name="b2fb")
        tmp_m1 = sb.tile([128, KM1], F32, name="tmp_m1")
        nc.vector.tensor_scalar(mean_p1[:], g2[:, 0:2], 1.0 / NG1, None, OP.mult)
        nc.vector.tensor_scalar(var_p1[:], g2[:, 2:4], 1.0 / NG1, None, OP.mult)
        nc.vector.tensor_tensor(tmp_m1[:], mean_p1[:], mean_p1[:], OP.mult)
        nc.vector.tensor_tensor(var_p1[:], var_p1[:], tmp_m1[:], OP.subtract)
        nc.vector.tensor_tensor(tmp_m1[:], a1c, a1c, OP.mult)
        nc.vector.tensor_tensor(var_p1[:], var_p1[:], tmp_m1[:], OP.mult)
        rsqrt_cols(a2f[:], var_p1[:], bn2g, extra_mul=a1c)

        # fold a2f into t2 weights; fill p1 pads with v2 — these unblock
        # the conv2 matmuls, so they come before the rest of the col math
        for k in range(KM1):
            nc.vector.tensor_scalar(
                t2s[k][:].rearrange("p t o -> p (t o)"),
                t2s[k][:].rearrange("p t o -> p (t o)"),
                a2f[:, k:k + 1], None, OP.mult)
        # v2 = -b2f/a2f = mean_p1 - bn2b/a2f
        nc.vector.reciprocal(tmp_m1[:], a2f[:])
        nc.vector.tensor_tensor(tmp_m1[:], bn2b, tmp_m1[:], OP.mult)
        nc.vector.tensor_tensor(v2[:], mean_p1[:], tmp_m1[:], OP.subtract)
        for m in range(KM1):
            pv = p1[m][:].rearrange("p (i r c) -> p i r c", i=NI, r=PW, c=PW)
            nc.vector.tensor_copy(
                pv[:, :, 0, :], v2[:, m:m + 1].broadcast_to([128, NI, PW]))
            nc.vector.tensor_copy(
                pv[:, :, 1:PW, 0], v2[:, m:m + 1].broadcast_to([128, NI, 28]))

        nc.vector.tensor_tensor(tmp_m1[:], a2f[:], mean_p1[:], OP.mult)
        nc.vector.tensor_tensor(b2f[:], bn2b, tmp_m1[:], OP.subtract)
        nc.vector.tensor_copy(b2fb[:], b2f[:])

        # ds BN affine (uses AR2 q stats; needed only for the q affine)
        mean_q = sb.tile([128, KM2], F32, name="mean_q")
        var_q = sb.tile([128, KM2], F32, name="var_q")
        aq = sb.tile([128, KM2], F32, name="aq")
        bq = sb.tile([128, KM2], F32, name="bq")
        tmp_m2 = sb.tile([128, KM2], F32, name="tmp_m2")
        nc.vector.tensor_scalar(mean_q[:], g2[:, 4:12], 1.0 / NG2, None, OP.mult)
        nc.vector.tensor_scalar(var_q[:], g2[:, 12:20], 1.0 / NG2, None, OP.mult)
        nc.vector.tensor_tensor(tmp_m2[:], mean_q[:], mean_q[:], OP.mult)
        nc.vector.tensor_tensor(var_q[:], var_q[:], tmp_m2[:], OP.subtract)
        rsqrt_cols(aq[:], var_q[:], dsg)
        nc.vector.tensor_tensor(tmp_m2[:], aq[:], mean_q[:], OP.mult)
        nc.vector.tensor_tensor(bq[:], dsb, tmp_m2[:], OP.subtract)

        # T[o] = sum_i S2sum[i,o] * b2f[i]  (border-free BN2 shift)
        Tc = sb.tile([128, KM1], F32, name="Tc")
        for m in range(KM1):
            tps = ps_b.tile([128, NCH], F32, tag="c2", name=f"tps_{m}")
            for k in range(KM1):
                nc.tensor.matmul(tps[:, 0:1], s2s[k][:, m * 128:(m + 1) * 128],
                                 b2fb[:, k:k + 1],
                                 start=(k == 0), stop=(k == KM1 - 1))
            nc.vector.tensor_copy(Tc[:, m:m + 1], tps[:, 0:1])

        # ================= conv2: 3x3 s2 p1 from padded p1 =================
        p1v = [p1[k][:].rearrange("p (i r c) -> p i r c", i=NI, r=PW, c=PW)
               for k in range(KM1)]
        for m in range(KM1):
            pmms = [ps_b.tile([128, NCH], F32, tag="c2",
                              name=f"psc2_{m}_{ip}") for ip in range(4)]
            for t, (dy, dx) in enumerate(TAPS):
                for k in range(KM1):
                    for ip in range(4):
                        rhs = p1v[k][:, 2 * ip:2 * ip + 2,
                                     dy:dy + 27:2, dx:dx + 27:2]
                        nc.tensor.matmul(
                            pmms[ip][:],
                            t2s[k][:, t, m * 128:(m + 1) * 128],
                            rhs, start=(t == 0 and k == 0),
                            stop=(t == 8 and k == KM1 - 1))
            for ip in range(4):
                slot = m * 4 + ip
                # moments of the pre-bias psum (Tc correction applied to
                # the mean after AR3); copy adds the Tc shift
                nc.vector.bn_stats(p2bn[:, slot * 6:(slot + 1) * 6],
                                   pmms[ip][:])
                nc.vector.tensor_scalar(
                    p2[m][:, ip * NCH:(ip + 1) * NCH], pmms[ip][:],
                    1.0, Tc[:, m:m + 1], OP.mult, OP.add)

        # p2 stats (pre-bias moments) -> st3 -> AR3
        for m in range(KM1):
            nc.vector.bn_aggr(
                p2agg[:, m * 2:m * 2 + 2],
                p2bn[:, m * 24:(m + 1) * 24].rearrange("p (g s) -> p g s",
                                                       s=6))
        p2av = p2agg[:].rearrange("p (m two) -> p m two", two=2)
        p2msq = sb.tile([128, KM1], F32, name="p2msq")
        nc.vector.tensor_tensor(p2msq[:], p2av[:, :, 0], p2av[:, :, 0],
                                OP.mult)
        nc.vector.tensor_scalar(st3[:, 0:KM1], p2av[:, :, 0], float(PIX2),
                                None, OP.mult)
        nc.vector.tensor_tensor(p2msq[:], p2msq[:], p2av[:, :, 1], OP.add)
        nc.vector.tensor_scalar(st3[:, 2:4], p2msq[:], float(PIX2),
                                None, OP.mult)
        ar3_in = dram.tile([128, 4], F32, name="ar3_in")
        ar3_out = dram.tile([128, 4], F32, name="ar3_out")
        nc.sync.dma_start(ar3_in[:], st3[:])
        nc.gpsimd.collective_compute(
            "AllReduce", OP.add, replica_groups=[list(range(NCORES))],
            ins=[ar3_in.opt()], outs=[ar3_out.opt()])
        nc.sync.dma_start(g3[:], ar3_out[:])

        # shortcut affine in place on q — overlaps AR3 (needs only AR2 stats)
        for m in range(KM2):
            nc.vector.tensor_scalar(q[m][:], q[m][:], aq[:, m:m + 1],
                                    bq[:, m:m + 1], OP.mult, OP.add)

        # ---- BN3 affine (alpha2-corrected) -> xn3 in place on p2 ----
        mean_p2 = sb.tile([128, KM1], F32, name="mean_p2")
        var_p2 = sb.tile([128, KM1], F32, name="var_p2")
        a3f = sb.tile([128, KM1], F32, name="a3f")
        b3f = sb.tile([128, KM1], F32, name="b3f")
        tmp_m3 = sb.tile([128, KM1], F32, name="tmp_m3")
        nc.vector.tensor_scalar(mean_p2[:], g3[:, 0:2], 1.0 / NG2, None, OP.mult)
        nc.vector.tensor_scalar(var_p2[:], g3[:, 2:4], 1.0 / NG2, None, OP.mult)
        nc.vector.tensor_tensor(tmp_m3[:], mean_p2[:], mean_p2[:], OP.mult)
        nc.vector.tensor_tensor(var_p2[:], var_p2[:], tmp_m3[:], OP.subtract)
        # stats were taken pre-bias; the copies added Tc, so shift the mean
        nc.vector.tensor_tensor(mean_p2[:], mean_p2[:], Tc[:], OP.add)
        nc.vector.tensor_tensor(tmp_m3[:], a2c, a2c, OP.mult)
        nc.vector.tensor_tensor(var_p2[:], var_p2[:], tmp_m3[:], OP.mult)
        rsqrt_cols(a3f[:], var_p2[:], bn3g, extra_mul=a2c)
        nc.vector.tensor_tensor(tmp_m3[:], a3f[:], mean_p2[:], OP.mult)
        nc.vector.tensor_tensor(b3f[:], bn3b, tmp_m3[:], OP.subtract)
        for m in range(KM1):
            nc.vector.tensor_scalar(p2[m][:], p2[m][:], a3f[:, m:m + 1],
                                    b3f[:, m:m + 1], OP.mult, OP.add)

        # ================= conv3 + residual fuse + store =================
        for m in range(KM2):
            for j in range(4):
                pmm = ps_a.tile([128, NCH], F32, tag="mm",
                                name=f"psc3_{m}_{j}")
                for k in range(KM1):
                    nc.tensor.matmul(
                        pmm[:], t3s[k][:, m * 128:(m + 1) * 128],
                        p2[k][:, j * NCH:(j + 1) * NCH],
                        start=(k == 0), stop=(k == KM1 - 1))
                out_t = sb_out.tile([128, NCH], BF16, tag="out",
                                    name=f"out_{m}_{j}")
                qc = q[m][:, j * NCH:(j + 1) * NCH]
                if j == 0:
                    nc.vector.scalar_tensor_tensor(
                        out_t[:], pmm[:], a3c[:, m:m + 1], qc,
                        OP.mult, OP.add)
                else:
                    # ACT scales out of psum, DVE adds the shortcut (2x)
                    nc.scalar.activation(out_t[:], pmm[:], ACT.Copy,
                                         scale=a3c[:, m:m + 1])
                    nc.vector.tensor_tensor(out_t[:], out_t[:], qc, OP.add)
                nc.sync.dma_start(
                    out_d.ap()[m * 128:(m + 1) * 128,
                               j * NCH:(j + 1) * NCH],
                    out_t[:])


def _prep_host(inputs):
    """Host-side: shard x, fold weights, build per-core in_maps."""
    x = np.asarray(inputs["x"], np.float32)          # [64, 512, 28, 28]
    w1 = np.asarray(inputs["w1"], np.float32)
    w2 = np.asarray(inputs["w2"], np.float32)
    w3 = np.asarray(inputs["w3"], np.float32)
    ds_w = np.asarray(inputs["ds_w"], np.float32)

    s1, al1 = _ternarize_host(w1)    # [256,512,1,1]
    s2, al2 = _ternarize_host(w2)    # [256,256,3,3]
    s3, al3 = _ternarize_host(w3)    # [1024,256,1,1]

    t1 = np.ascontiguousarray(s1[:, :, 0, 0].T).astype(BF)       # [512, 256]
    # t2[t, i, o] = s2[o, i, ky, kx], t = ky*3+kx; shipped pre-tiled as
    # [k, i_local, t*C1 + o] so each partition row is one contiguous DMA
    t2 = s2.transpose(2, 3, 1, 0).reshape(9, C1, C1)
    s2sum = t2.sum(axis=0).astype(BF)                            # [256, 256]
    t2k = np.ascontiguousarray(
        t2.transpose(1, 0, 2).reshape(KM1, 128, 9 * C1)).astype(BF)
    t3 = np.ascontiguousarray(s3[:, :, 0, 0].T).astype(BF)       # [256, 1024]
    dsw = np.ascontiguousarray(ds_w[:, :, 0, 0].T).astype(BF)    # [512, 1024]

    def cols(v, n):
        # [n*128] channel vector -> [128, n] column layout
        return np.asarray(v, np.float32).reshape(n, 128).T

    par = np.concatenate([
        cols(al1, 2), cols(al2, 2), cols(al3, 8),
        cols(inputs["bn1_g"], 4), cols(inputs["bn2_g"], 2),
        cols(inputs["bn2_b"], 2), cols(inputs["bn3_g"], 2),
        cols(inputs["bn3_b"], 2), cols(inputs["ds_bn_g"], 8),
        cols(inputs["ds_bn_b"], 8),
    ], axis=1)
    par = np.ascontiguousarray(par, dtype=np.float32)            # [128, 40]

    common = dict(t1=t1, t2=t2k, s2sum=s2sum, t3=t3, dsw=dsw, par=par)

    in_maps = []
    for c in range(NCORES):
        xc = x[c * NI:(c + 1) * NI]                      # [8, 512, 28, 28]
        xc = np.ascontiguousarray(
            xc.transpose(1, 0, 2, 3).reshape(C0, PIX1)).astype(BF)
        in_maps.append({"x": xc, **common})
    return in_maps


def kernel(**inputs):
    if "nc" not in _CACHE:
        _CACHE["nc"] = build_program()
    nc = _CACHE["nc"]

    in_maps = _prep_host(inputs)
    try:
        res = run_bass_kernel_spmd(nc, in_maps, core_ids=list(range(NCORES)))
    except Exception:
        # transient device state (e.g. a previous crashed run) usually
        # clears on retry
        res = run_bass_kernel_spmd(nc, in_maps, core_ids=list(range(NCORES)))

    out = np.empty((64, C2, H2, H2), np.float32)
    for c in range(NCORES):
        oc = np.asarray(res.results[c]["out"]).astype(np.float32)
        oc = oc.reshape(C2, NI, H2, H2)
        out[c * NI:(c + 1) * NI] = oc.transpose(1, 0, 2, 3)
    return out
